# revision 28
# baseline (speedup 1.0000x reference)
"""Trainium2 Bass kernel for nn_Metalayer_sub_62869731279045.

Math: the oracle's edge list is the structured 1-D KNN=2 neighbor graph, so
C = I + Delta and Km are pentadiagonal.  Let D = -Delta and

  G  = wh * (B C + K)            (pentadiagonal, row-diagonals g_o)
  Ninv ~= I + D + D^2 + D^3      (Neumann, band 6)
  T  = Ninv * G - theta*I        (band 8, 17 diagonals, built on device
                                  via banded matrix-matrix products)

Since T is REAL, the expm action needs only a real Taylor chain:
  tau_k = T^k u0,   Uz = e^{i theta} * DX * sum_k (i^k / k!) tau_k
with i^k folded into which accumulator (s_re / s_im) receives each term.
KT=5 terms + fp16 MLP staging give ~1.2e-3 max-rel error vs fp64.

Layouts: length-2048 row vectors live as [128, 16] "fm" tiles (i = 16p+f).
Banded matvec = halo exchange via 2 PE shift-matmuls + one DVE windowed
multiply against 17 stacked diagonal planes + one segmented reduce.
Banded products for the operator assembly use the same windowed-multiply
trick over zero-padded plane tiles (all strides positive by storing the
5-wide D/G plane stacks in reversed diagonal order).

MLPs: c&k edge MLPs run as one 128-wide hidden pipeline over 8192 stacked
edge columns (4 bands); n&e node MLPs as one 128-wide pipeline whose last
layer emits Bd and all 32 Eys rows from a single [128,34] matmul.  Row ->
fm transposes bounce through DRAM with 64B-run descriptor patterns.

All 8 cores run the same single-core program on identical inputs (the
chain is a serial dependency; collectives cost more than they save).
Core 0's output is returned.
"""

import os
import sys
import numpy as np

for _p in ("/opt/trn_rl_repo",):
    if _p not in sys.path:
        sys.path.insert(0, _p)

N = 2048
RES = 32
H = 64
E = 8186
K_WAVE = 2.0 * np.pi / 1.55
WH = 0.75
DX = 1.0 / RES
THETA = 6.234
KT = 3    # Taylor terms (theta-shifted operator norm ~0.2; 3 suffices)
# band order for the stacked ck pipeline: o descending (matches reversed
# diagonal-plane storage so all product access patterns have +strides)
BAND_ORDER = [2, 1, -1, -2]
# (offset o, first valid row i0, edge count L, edge-array start e0)
BANDS = {-2: (2, 2046, 0), -1: (1, 2047, 2046), 1: (0, 2047, 4093), 2: (0, 2046, 6140)}

_CACHE = {}


def _build():
    from contextlib import ExitStack

    import concourse.bass as bass
    import concourse.mybir as mybir
    from concourse import bacc, tile

    f32 = mybir.dt.float32
    f16 = mybir.dt.float16
    AF = mybir.ActivationFunctionType
    ALU = mybir.AluOpType

    nc = bacc.Bacc("TRN2", target_bir_lowering=False, debug=False, num_devices=8)

    def Par(name, shape, dt=f32):
        return nc.declare_dram_parameter(name, list(shape), dt, isOutput=False)

    # consolidated input blobs (3 DMA loads total)
    # blob16 [3, 10496] f16: xt(0:8192) | hs row0 (8192:10240) |
    #   W1ne row0 (10240:10368) | W1ck (10368:10496)
    # blob32 [128, 871] f32: sdn(0:128) | sup(128:256) | bmask(256:320) |
    #   b1ne b2ne b1ck b2ck (320:324) | b3ck(324:326) | nb3(326) |
    #   eb3(327:359) | e0c(359:871)
    # blobw16 [128, 420] f16: W2ne(0:128) | W3neA(128:192) | W3neB(192:226) |
    #   W2ck(226:354) | W3ckA(354:418) | W3ckB(418:420)
    # The A variants are zero-padded to 64 out-partitions so the stacked l3
    # matmul pair covers psum rows 0..98 / 0..66 with no gap or overlap.
    bias8_d = Par("bias8", [128, 8])
    blob16_d = Par("blob16", [3, 10496], f16)
    blob32_d = Par("blob32", [128, 871])
    blobw16_d = Par("blobw16", [128, 420], f16)
    scratch = nc.dram_tensor("scratch", [43 * N], f32)
    out_d = nc.declare_dram_parameter("out", [N * RES, 2], f32, isOutput=True)
    debug = os.environ.get("KERNEL_DEBUG", "0") == "1"
    dbg_t = {}
    if debug:
        for nm, shape in [
            ("d_ckbdfm", [128, 144]),
            ("d_eys2b", [128, 512]), ("d_u0", [128, 16]),
            ("d_Dt", [128, 80]), ("d_Gt", [128, 200]), ("d_N1p", [128, 260]),
            ("d_N2p", [128, 340]), ("d_N3p", [128, 420]), ("d_Tpl", [128, 272]),
            ("d_sre", [128, 16]), ("d_sim", [128, 16]),
        ]:
            dbg_t[nm] = nc.dram_tensor(nm, shape, f32)

    TS = bass.ts

    def emit(tc, ctx, pools):
        (consts, work, vec, psA, psC) = pools

        def dma(out_ap, in_ap, eng=None):
            (eng or nc.sync).dma_start(out_ap, in_ap)

        # ---------------- consolidated input loads (3 DMAs) ----------------
        blob16 = consts.tile([3, 10496], f16, tag="blob16")
        bias8 = consts.tile([128, 8], f32, tag="bias8")
        blob32 = consts.tile([128, 871], f32, tag="blob32")
        blobw16 = consts.tile([128, 420], f16, tag="blobw16")
        dma(blob16[:], blob16_d[:], eng=nc.sync)
        dma(bias8[:], bias8_d[:], eng=nc.gpsimd)
        dma(blobw16[:], blobw16_d[:], eng=nc.sync)
        dma(blob32[:], blob32_d[:], eng=nc.gpsimd)
        B32W = 871
        BM = 256  # bmask col base in blob32

        def xt16(q):
            return blob16[:, TS(q, 512)]

        def hs16(q):
            return blob16[0:1, 8192 + q * 512 : 8192 + (q + 1) * 512]

        W1ne = blob16[0:1, 10240:10368]
        W1ck = blob16[:, 10368:10496]
        W2ne = blobw16[:, 0:128]
        W3neA = blobw16[:, 128:192]
        W3neB = blobw16[:, 192:226]
        W2ck = blobw16[:, 226:354]
        W3ckA = blobw16[:, 354:418]
        W3ckB = blobw16[:, 418:420]
        sdn = blob32[:, 0:128]
        sup = blob32[:, 128:256]
        bmask = blob32[:, 256:320]
        b1ne = bias8[:, 0:1]
        b2ne = bias8[:, 1:2]
        b1ck = bias8[:, 2:3]
        b2ck = bias8[:, 3:4]
        b3c = bias8[:, 4:5]
        b3k = bias8[:, 5:6]
        nb3 = bias8[:, 6:7]

        # ---------------- persistent SBUF tiles ----------------
        h1ne = consts.tile([128, N], f16, tag="h1ne")
        h2ne = consts.tile([128, N], f16, tag="h2ne")
        rows_neS = consts.tile([98, N // 2], f32, tag="rows_neS")
        h1ck = consts.tile([128, 4 * N], f16, tag="h1ck")
        h2ck = consts.tile([128, 4 * N], f16, tag="h2ck")
        rows_ckS = consts.tile([66, 4096], f32, tag="rows_ckS")
        ckbdfm = consts.tile([128, 144], f32, tag="ckbdfm")
        eys2 = consts.tile([128, 16 * RES], f32, tag="eys2")
        eys2b = consts.tile([128, 16 * RES], f32, tag="eys2b")
        u0 = consts.tile([128, 16], f32, tag="u0")
        Dt = consts.tile([128, 80], f32, tag="Dt")        # 5 planes x 16, rev
        Gt = consts.tile([128, 200], f32, tag="Gt")       # 5 planes x 40, rev
        N1p = consts.tile([128, 260], f32, tag="N1p")     # 13 planes x 20
        N2p = consts.tile([128, 340], f32, tag="N2p")     # 17 planes x 20
        N3p = consts.tile([128, 420], f32, tag="N3p")     # 21 planes x 20
        Tpl = consts.tile([128, 272], f32, tag="Tpl")     # col = f*17 + s
        s_re = consts.tile([128, 16], f32, tag="s_re")
        s_im = consts.tile([128, 16], f32, tag="s_im")
        o_int = consts.tile([128, 2 * 16 * RES], f32, tag="o_int")

        AP = bass.AP

        def ap(t, off, dims):
            return AP(t.tensor, t.offset + off, dims)

        tanhc = work.tile([128, 64], f32, tag="tanhc")
        tanhk = work.tile([128, 64], f32, tag="tanhk")
        tb = work.tile([128, 16], f32, tag="tb")
        Bdp = work.tile([128, 16], f32, tag="Bdp")
        Bdp01 = work.tile([128, 16], f32, tag="Bdp01")

        # early memsets of padded plane tiles (no deps -> overlap with MLPs)
        nc.vector.memset(Gt[:], 0.0)
        nc.gpsimd.memset(N1p[:], 0.0)
        nc.vector.memset(N2p[:], 0.0)
        nc.gpsimd.memset(N3p[:], 0.0)
        nc.vector.memset(Dt[:, 32:48], 0.0)
        nc.gpsimd.memset(s_im[:], 0.0)
        nc.gpsimd.memset(ckbdfm[:], 0.0)

        # ------- fused ck + ne pipelines (ne chunks interleaved as filler) ----
        # ck: c & k edge MLPs over 4*2048 stacked band columns; ne: n & e node
        # MLPs over 2048 columns.  l3s are partition-stacked (PE psum base
        # must be 0/32/64) so one copy retires two chunks.
        def ck_iter(r):
            ps1 = psA.tile([128, 1024], f32, tag="psA")
            nc.tensor.matmul(ps1[:, 0:512], W1ck, xt16(2 * r))
            nc.tensor.matmul(ps1[:, 512:1024], W1ck, xt16(2 * r + 1))
            nc.scalar.activation(
                h1ck[:, TS(r, 1024)], ps1[:], AF.Relu, bias=b1ck
            )
            ps2 = psA.tile([128, 1024], f32, tag="psA")
            nc.tensor.matmul(ps2[:, 0:512], W2ck, h1ck[:, TS(2 * r, 512)])
            nc.tensor.matmul(ps2[:, 512:1024], W2ck, h1ck[:, TS(2 * r + 1, 512)])
            nc.vector.tensor_scalar(
                h2ck[:, TS(r, 1024)], ps2[:], b2ck, 0.0, ALU.add, op1=ALU.max
            )
            ps3 = psC.tile([98, 512], f32, tag="psC")
            nc.tensor.matmul(ps3[0:64, :], W3ckA, h2ck[:, TS(2 * r, 512)])
            nc.tensor.matmul(
                ps3[64:66, :], W3ckB, h2ck[:, TS(2 * r + 1, 512)]
            )
            if r % 2 == 0:
                nc.scalar.activation(rows_ckS[:, TS(r, 512)], ps3[0:66, :], AF.Copy)
            else:
                nc.vector.tensor_copy(rows_ckS[:, TS(r, 512)], ps3[0:66, :])

        def ne_iter(r):
            ps1 = psA.tile([128, 1024], f32, tag="psA")
            nc.tensor.matmul(ps1[:, 0:512], W1ne, hs16(2 * r))
            nc.tensor.matmul(ps1[:, 512:1024], W1ne, hs16(2 * r + 1))
            nc.scalar.activation(
                h1ne[:, TS(r, 1024)], ps1[:], AF.Relu, bias=b1ne
            )
            ps2 = psA.tile([128, 1024], f32, tag="psA")
            nc.tensor.matmul(ps2[:, 0:512], W2ne, h1ne[:, TS(2 * r, 512)])
            nc.tensor.matmul(ps2[:, 512:1024], W2ne, h1ne[:, TS(2 * r + 1, 512)])
            nc.vector.tensor_scalar(
                h2ne[:, TS(r, 1024)], ps2[:], b2ne, 0.0, ALU.add, op1=ALU.max
            )
            ps3 = psC.tile([98, 512], f32, tag="psC")
            nc.tensor.matmul(ps3[0:64, :], W3neA, h2ne[:, TS(2 * r, 512)])
            nc.tensor.matmul(
                ps3[64:98, :], W3neB, h2ne[:, TS(2 * r + 1, 512)]
            )
            nc.scalar.activation(rows_neS[:, TS(r, 512)], ps3[:], AF.Copy)

        # half-bounce: write bands 2u, 2u+1 (t blocks 4u..4u+3) to scratch in
        # group-major order, read them back as fm grps, and tanh them.
        # value(ck, band b, n') at scratch[(1+4ck+b)*2048 + n'] comes from
        # rows_ckS[64h+ck, 512t+j] with n = 512(2t+h)+j = 2048b + n'.
        def ck_bounce(u):
            for h in range(2):
                dma(
                    AP(
                        scratch,
                        N + 4096 * u + 512 * h,
                        [[8192, 2], [2048, 2], [1024, 2], [1, 512]],
                    ),
                    ap(
                        rows_ckS,
                        64 * h * 4096 + 2048 * u,
                        [[4096, 2], [1024, 2], [512, 2], [1, 512]],
                    ),
                    eng=nc.sync if h == 0 else nc.gpsimd,
                )
            for ck in range(2):
                dma(
                    ap(ckbdfm, 16 + 64 * ck + 32 * u, [[144, 128], [16, 2], [1, 16]]),
                    AP(
                        scratch,
                        N + 8192 * ck + 4096 * u,
                        [[16, 128], [2048, 2], [1, 16]],
                    ),
                    eng=nc.sync if ck == 0 else nc.gpsimd,
                )
            nc.scalar.activation(
                tanhc[:, 32 * u : 32 * u + 32],
                ckbdfm[:, 16 + 32 * u : 48 + 32 * u],
                AF.Tanh,
                bias=b3c,
            )
            nc.scalar.activation(
                tanhk[:, 32 * u : 32 * u + 32],
                ckbdfm[:, 80 + 32 * u : 112 + 32 * u],
                AF.Tanh,
                bias=b3k,
            )
            # D planes for this half (reversed: plane j <-> o1 = 2-j) and the
            # matching ascending N1 = I + D planes
            nc.vector.scalar_tensor_tensor(
                ap(Dt, 48 * u, [[80, 128], [16, 2], [1, 16]]),
                ap(tanhc, 32 * u, [[64, 128], [16, 2], [1, 16]]),
                -0.1,
                ap(blob32, BM + 32 * u, [[B32W, 128], [16, 2], [1, 16]]),
                ALU.mult,
                ALU.mult,
            )
            for j, b0 in ((8, 0), (7, 16)) if u == 0 else ((5, 32), (4, 48)):
                nc.vector.scalar_tensor_tensor(
                    ap(N1p, j * 20 + 2, [[260, 128], [1, 16]]),
                    ap(tanhc, b0, [[64, 128], [1, 16]]),
                    -0.1,
                    ap(blob32, BM + b0, [[B32W, 128], [1, 16]]),
                    ALU.mult,
                    ALU.mult,
                )

        def ck_bounce_band2():
            # band 2 (o=-1) alone: t blocks 4,5
            for h in range(2):
                dma(
                    AP(scratch, N + 2 * 2048 + 512 * h, [[8192, 2], [1024, 2], [1, 512]]),
                    ap(rows_ckS, 64 * h * 4096 + 2048, [[4096, 2], [512, 2], [1, 512]]),
                    eng=nc.sync if h == 0 else nc.gpsimd,
                )
            for ck in range(2):
                dma(
                    ap(ckbdfm, 16 + 64 * ck + 32, [[144, 128], [1, 16]]),
                    AP(scratch, N + (2 + 4 * ck) * 2048, [[16, 128], [1, 16]]),
                    eng=nc.sync if ck == 0 else nc.gpsimd,
                )
            nc.scalar.activation(
                tanhc[:, 32:48], ckbdfm[:, 48:64], AF.Tanh, bias=b3c
            )
            nc.scalar.activation(
                tanhk[:, 32:48], ckbdfm[:, 112:128], AF.Tanh, bias=b3k
            )
            nc.vector.scalar_tensor_tensor(
                ap(Dt, 48, [[80, 128], [1, 16]]),
                ap(tanhc, 32, [[64, 128], [1, 16]]),
                -0.1,
                ap(blob32, BM + 32, [[B32W, 128], [1, 16]]),
                ALU.mult,
                ALU.mult,
            )
            nc.vector.scalar_tensor_tensor(
                ap(N1p, 5 * 20 + 2, [[260, 128], [1, 16]]),
                ap(tanhc, 32, [[64, 128], [1, 16]]),
                -0.1,
                ap(blob32, BM + 32, [[B32W, 128], [1, 16]]),
                ALU.mult,
                ALU.mult,
            )

        def ck_band3_direct():
            # band 3 (o=-2): skip the DRAM bounce; build the fm tile straight
            # from h2ck with 16 strided-lhsT matmuls (edge-group -> partition).
            # psum col 2f+ck = value(ck, band3, 16p+f)
            psB3 = psC.tile([128, 32], f32, tag="psC")
            for fcol in range(16):
                nc.tensor.matmul(
                    psB3[:, 2 * fcol : 2 * fcol + 2],
                    ap(h2ck, 3 * 2048 + fcol, [[8192, 128], [16, 128]]),
                    W3ckB,
                )
            nc.scalar.activation(
                tanhc[:, 48:64],
                ap(psB3, 0, [[32, 128], [2, 16]]),
                AF.Tanh,
                bias=b3c,
            )
            nc.scalar.activation(
                tanhk[:, 48:64],
                ap(psB3, 1, [[32, 128], [2, 16]]),
                AF.Tanh,
                bias=b3k,
            )
            nc.vector.scalar_tensor_tensor(
                ap(Dt, 64, [[80, 128], [1, 16]]),
                ap(tanhc, 48, [[64, 128], [1, 16]]),
                -0.1,
                ap(blob32, BM + 48, [[B32W, 128], [1, 16]]),
                ALU.mult,
                ALU.mult,
            )
            nc.vector.scalar_tensor_tensor(
                ap(N1p, 4 * 20 + 2, [[260, 128], [1, 16]]),
                ap(tanhc, 48, [[64, 128], [1, 16]]),
                -0.1,
                ap(blob32, BM + 48, [[B32W, 128], [1, 16]]),
                ALU.mult,
                ALU.mult,
            )

        def ne_writes():
            # ne rows -> DRAM scratch (0:2048 Bd_pre; 11*N.. eys r-major);
            # value of node n = 512*(2a+h)+j lives at rows_neS[64h+row, 512a+j].
            # All on the gpsimd queue so they never sit behind ck bounces.
            for h in range(2):
                dma(
                    AP(scratch, 512 * h, [[1024, 2], [1, 512]]),
                    ap(rows_neS, 64 * h * 1024, [[1024, 1], [512, 2], [1, 512]]),
                    eng=nc.gpsimd,
                )
                dma(
                    AP(scratch, 11 * N + 512 * h, [[2048, 32], [1024, 2], [1, 512]]),
                    ap(rows_neS, (64 * h + 2) * 1024, [[1024, 32], [512, 2], [1, 512]]),
                    eng=nc.gpsimd,
                )
            dma(
                ap(ckbdfm, 0, [[144, 128], [1, 16]]),
                AP(scratch, 0, [[16, 128], [1, 16]]),
                eng=nc.gpsimd,
            )

        ck_iter(0)
        ck_iter(1)
        ne_iter(0)
        ck_iter(2)
        ck_iter(3)
        ck_bounce(0)
        ck_iter(4)
        ne_iter(1)
        ne_writes()
        ck_iter(5)
        ck_bounce_band2()
        ck_iter(6)
        ck_iter(7)
        ck_band3_direct()
        # eys2[p, r*16+f] = eys[16p+f, r]
        dma(
            ap(eys2, 0, [[512, 128], [16, 32], [1, 16]]),
            AP(scratch, 11 * N, [[16, 128], [2048, 32], [1, 16]]),
        )
        # ---------------- diagonal planes ----------------
        nc.scalar.activation(tb[:], ckbdfm[:, 0:16], AF.Tanh, bias=nb3)
        # Bd' = wh*K*(2 + 0.5*tanh)
        nc.vector.tensor_scalar(
            Bdp[:], tb[:], 0.5 * K_WAVE * WH, 2.0 * K_WAVE * WH, ALU.mult, op1=ALU.add
        )
        nc.vector.tensor_scalar(Bdp01[:], Bdp[:], 0.1, 0.0, ALU.mult, op1=ALU.add)

        # G planes (reversed, width 40 = 12|16|12), g = (0.1*Bd'*tc + ck2*tk)*mask
        gm4 = work.tile([128, 64], f32, tag="gm4")
        g4 = work.tile([128, 64], f32, tag="g4")
        nc.vector.tensor_tensor(
            ap(gm4, 0, [[64, 128], [16, 4], [1, 16]]),
            ap(tanhc, 0, [[64, 128], [16, 4], [1, 16]]),
            ap(Bdp01, 0, [[16, 128], [0, 4], [1, 16]]),
            ALU.mult,
        )
        nc.vector.scalar_tensor_tensor(
            g4[:], tanhk[:], 0.1 * K_WAVE * WH, gm4[:], ALU.mult, ALU.add
        )
        nc.vector.tensor_tensor(
            ap(Gt, 12, [[200, 128], [40, 2], [1, 16]]),
            ap(g4, 0, [[64, 128], [16, 2], [1, 16]]),
            ap(blob32, BM, [[B32W, 128], [16, 2], [1, 16]]),
            ALU.mult,
        )
        nc.vector.tensor_tensor(
            ap(Gt, 3 * 40 + 12, [[200, 128], [40, 2], [1, 16]]),
            ap(g4, 32, [[64, 128], [16, 2], [1, 16]]),
            ap(blob32, BM + 32, [[B32W, 128], [16, 2], [1, 16]]),
            ALU.mult,
        )
        nc.vector.tensor_copy(Gt[:, 2 * 40 + 12 : 2 * 40 + 28], Bdp[:])
        nc.vector.memset(N1p[:, 6 * 20 + 2 : 6 * 20 + 18], 1.0)

        # halo fill helper: data planes [first..first+n) of a padded tile,
        # plane width w, data at col `pad` of each plane, fill width hw <= pad.
        def halo(t, first, nplanes, w, pad, hw):
            ps = psC.tile([128, 2 * nplanes * hw], f32, tag="psC")
            base = first * w + pad
            # left pads <- v[p-1, f in 16-hw..16] (sup)
            nc.tensor.matmul(
                ps[:, 0 : nplanes * hw],
                sup,
                ap(t, base + 16 - hw, [[t.shape[1], 128], [w, nplanes], [1, hw]]),
            )
            # right pads <- v[p+1, f in 0..hw] (sdn)
            nc.tensor.matmul(
                ps[:, nplanes * hw : 2 * nplanes * hw],
                sdn,
                ap(t, base, [[t.shape[1], 128], [w, nplanes], [1, hw]]),
            )
            nc.vector.tensor_copy(
                ap(
                    t,
                    base - hw,
                    [[t.shape[1], 128], [16 + hw, 2], [w, nplanes], [1, hw]],
                ),
                ap(
                    ps,
                    0,
                    [[2 * nplanes * hw, 128], [nplanes * hw, 2], [hw, nplanes], [1, hw]],
                ),
            )

        halo(Gt, 0, 5, 40, 12, 6)
        halo(N1p, 4, 5, 20, 2, 2)

        # banded product: out_pad.data = D * in_pad  (planes ascending),
        # in0 = Dt (reversed), in1 = in_pad at addr 20*O + 19*j + f + 4.
        # Split over the j (diagonal) axis: DVE takes j 0:3, Pool j 3:5, then
        # DVE adds the Pool partial into the padded output.
        def dprod(in_pad, out_pad, NPo, tag):
            W_in, W_out = in_pad.shape[1], out_pad.shape[1]
            prD = work.tile([128, NPo * 48], f32, tag=tag + "D")
            prD_ap = ap(prD, 0, [[NPo * 48, 128], [48, NPo], [3, 16], [1, 3]])
            nc.vector.tensor_tensor(
                prD_ap,
                ap(Dt, 0, [[80, 128], [0, NPo], [1, 16], [16, 3]]),
                ap(in_pad, 4, [[W_in, 128], [20, NPo], [1, 16], [19, 3]]),
                ALU.mult,
            )
            prP = work.tile([128, NPo * 32], f32, tag=tag + "P")
            prP_ap = ap(prP, 0, [[NPo * 32, 128], [32, NPo], [2, 16], [1, 2]])
            nc.gpsimd.tensor_tensor(
                prP_ap,
                ap(Dt, 48, [[80, 128], [0, NPo], [1, 16], [16, 2]]),
                ap(in_pad, 61, [[W_in, 128], [20, NPo], [1, 16], [19, 2]]),
                ALU.mult,
            )
            tmpP = work.tile([128, NPo * 16], f32, tag=tag + "T")
            nc.gpsimd.tensor_tensor(
                ap(tmpP, 0, [[NPo * 16, 128], [16, NPo], [1, 16]]),
                ap(prP, 0, [[NPo * 32, 128], [32, NPo], [2, 16]]),
                ap(prP, 1, [[NPo * 32, 128], [32, NPo], [2, 16]]),
                ALU.add,
            )
            out_ap = ap(out_pad, 82, [[W_out, 128], [20, NPo], [1, 16]])
            nc.vector.reduce_sum(out_ap, prD_ap, axis=mybir.AxisListType.X)
            nc.vector.tensor_tensor(
                out_ap,
                out_ap,
                ap(tmpP, 0, [[NPo * 16, 128], [16, NPo], [1, 16]]),
                ALU.add,
            )

        dprod(N1p, N2p, 9, "pr2")
        nc.vector.tensor_scalar(
            N2p[:, 8 * 20 + 2 : 8 * 20 + 18],
            N2p[:, 8 * 20 + 2 : 8 * 20 + 18],
            1.0, 0.0, ALU.add, op1=ALU.add,
        )
        halo(N2p, 4, 9, 20, 2, 2)
        dprod(N2p, N3p, 13, "pr3")
        nc.vector.tensor_scalar(
            N3p[:, 10 * 20 + 2 : 10 * 20 + 18],
            N3p[:, 10 * 20 + 2 : 10 * 20 + 18],
            1.0, 0.0, ALU.add, op1=ALU.add,
        )
        # T = N3 * G - theta I   (no halo needed on N3p: read unshifted),
        # j-split DVE (j 0:3) / Pool (j 3:5) like dprod.
        prT = work.tile([128, 17 * 48], f32, tag="prTD")
        prT_ap = ap(prT, 0, [[17 * 48, 128], [48, 17], [3, 16], [1, 3]])
        nc.vector.tensor_tensor(
            prT_ap,
            ap(N3p, 2, [[420, 128], [20, 17], [1, 16], [20, 3]]),
            ap(Gt, 2, [[200, 128], [1, 17], [1, 16], [41, 3]]),
            ALU.mult,
        )
        prTP = work.tile([128, 17 * 32], f32, tag="prTP")
        prTP_ap = ap(prTP, 0, [[17 * 32, 128], [32, 17], [2, 16], [1, 2]])
        nc.gpsimd.tensor_tensor(
            prTP_ap,
            ap(N3p, 62, [[420, 128], [20, 17], [1, 16], [20, 2]]),
            ap(Gt, 125, [[200, 128], [1, 17], [1, 16], [41, 2]]),
            ALU.mult,
        )
        tmpTP = work.tile([128, 272], f32, tag="tmpTP")
        nc.gpsimd.tensor_tensor(
            ap(tmpTP, 0, [[272, 128], [16, 17], [1, 16]]),
            ap(prTP, 0, [[17 * 32, 128], [32, 17], [2, 16]]),
            ap(prTP, 1, [[17 * 32, 128], [32, 17], [2, 16]]),
            ALU.add,
        )
        TplOF = ap(Tpl, 0, [[272, 128], [1, 17], [17, 16]])
        nc.vector.reduce_sum(TplOF, prT_ap, axis=mybir.AxisListType.X)
        nc.vector.tensor_tensor(
            TplOF, TplOF, ap(tmpTP, 0, [[272, 128], [16, 17], [1, 16]]), ALU.add
        )
        nc.vector.tensor_scalar(
            ap(Tpl, 8, [[272, 128], [17, 16]]),
            ap(Tpl, 8, [[272, 128], [17, 16]]),
            -THETA, 0.0, ALU.add, op1=ALU.add,
        )

        # eys2b = eys2 + eb3 (per-r bias)
        nc.vector.tensor_tensor(
            ap(eys2b, 0, [[512, 128], [16, 32], [1, 16]]),
            ap(eys2, 0, [[512, 128], [16, 32], [1, 16]]),
            ap(blob32, 327, [[B32W, 128], [1, 32], [0, 16]]),
            ALU.add,
        )
        # u0[i] = sum_r eys2b[i,r] * e0c[i,r]
        pu = work.tile([128, 16 * RES], f32, tag="pu")
        nc.vector.tensor_tensor(
            ap(pu, 0, [[512, 128], [32, 16], [1, 32]]),
            ap(eys2b, 0, [[512, 128], [1, 16], [16, 32]]),
            ap(blob32, 359, [[B32W, 128], [32, 16], [1, 32]]),
            ALU.mult,
        )
        nc.vector.reduce_sum(
            u0[:],
            ap(pu, 0, [[512, 128], [32, 16], [1, 32]]),
            axis=mybir.AxisListType.X,
        )


        # ---------------- real Taylor chain ----------------
        t_cur = vec.tile([128, 32], f32, tag="vec")
        nc.vector.memset(t_cur[:], 0.0)
        nc.vector.tensor_copy(t_cur[:, 8:24], u0[:])
        nc.vector.tensor_scalar(
            s_re[:], u0[:], DX, 0.0, ALU.mult, op1=ALU.add
        )
        fact = 1.0
        for k in range(1, KT + 1):
            psh = psC.tile([128, 16], f32, tag="psC")
            nc.tensor.matmul(psh[:, 0:8], sup, t_cur[:, 16:24])
            nc.tensor.matmul(psh[:, 8:16], sdn, t_cur[:, 8:16])
            nc.vector.tensor_copy(
                ap(t_cur, 0, [[32, 128], [24, 2], [1, 8]]),
                ap(psh, 0, [[16, 128], [8, 2], [1, 8]]),
            )
            pr = work.tile([128, 272], f32, tag="prc")
            pr_ap = ap(pr, 0, [[272, 128], [17, 16], [1, 17]])
            nc.vector.tensor_tensor(
                pr_ap,
                ap(t_cur, 0, [[32, 128], [1, 16], [1, 17]]),
                ap(Tpl, 0, [[272, 128], [17, 16], [1, 17]]),
                ALU.mult,
            )
            t_nxt = vec.tile([128, 32], f32, tag="vec")
            nc.vector.reduce_sum(
                ap(t_nxt, 8, [[32, 128], [1, 16]]), pr_ap, axis=mybir.AxisListType.X
            )
            fact *= k
            coef = DX / fact * (-1.0 if k % 4 in (2, 3) else 1.0)
            dst = s_im if k % 2 == 1 else s_re
            nc.vector.scalar_tensor_tensor(
                dst[:], t_nxt[:, 8:24], coef, dst[:], ALU.mult, ALU.add
            )
            t_cur = t_nxt

        # ---------------- Uz = e^{i theta} s;  En = Uz * Eys ----------------
        cth, sth = float(np.cos(THETA)), float(np.sin(THETA))
        uzr = work.tile([128, 16], f32, tag="uzr")
        uzi = work.tile([128, 16], f32, tag="uzi")
        p1 = work.tile([128, 16], f32, tag="p1")
        nc.vector.tensor_scalar(p1[:], s_im[:], sth, 0.0, ALU.mult, op1=ALU.add)
        nc.vector.scalar_tensor_tensor(
            uzr[:], s_re[:], cth, p1[:], ALU.mult, ALU.subtract
        )
        nc.vector.tensor_scalar(p1[:], s_re[:], sth, 0.0, ALU.mult, op1=ALU.add)
        nc.vector.scalar_tensor_tensor(
            uzi[:], s_im[:], cth, p1[:], ALU.mult, ALU.add
        )
        if debug:
            for nm, t in [
                ("d_ckbdfm", ckbdfm), ("d_eys2b", eys2b),
                ("d_u0", u0), ("d_Dt", Dt), ("d_Gt", Gt), ("d_N1p", N1p),
                ("d_N2p", N2p), ("d_N3p", N3p), ("d_Tpl", Tpl), ("d_sre", s_re),
                ("d_sim", s_im),
            ]:
                nc.sync.dma_start(dbg_t[nm][:], t[:])
        # interleave En = Uz * Eys into o_int and stream out in two halves so
        # the first DMA overlaps the second half's compute
        for half in range(2):
            for c, uz in ((0, uzr), (1, uzi)):
                nc.vector.tensor_tensor(
                    ap(o_int, 512 * half + c, [[1024, 128], [64, 8], [2, 32]]),
                    ap(eys2b, 8 * half, [[512, 128], [1, 8], [16, 32]]),
                    ap(uz, 8 * half, [[16, 128], [1, 8], [0, 32]]),
                    ALU.mult,
                )
            (nc.sync if half == 0 else nc.gpsimd).dma_start(
                AP(out_d, 512 * half, [[1024, 128], [1, 512]]),
                o_int[:, TS(half, 512)],
            )

    with tile.TileContext(nc) as tc:
        ctx = ExitStack()
        try:
            pools = (
                ctx.enter_context(tc.tile_pool(name="consts", bufs=1)),
                ctx.enter_context(tc.tile_pool(name="work", bufs=2)),
                ctx.enter_context(tc.tile_pool(name="vec", bufs=3)),
                ctx.enter_context(tc.tile_pool(name="psA", bufs=3, space="PSUM")),
                ctx.enter_context(tc.tile_pool(name="psC", bufs=2, space="PSUM")),
            )
            emit(tc, ctx, pools)
        finally:
            ctx.close()

    nc.compile()
    nc.finalize()
    return nc


def _host_inputs(inputs):
    """Stage the oracle's inputs into the kernel's DRAM parameters."""
    f16 = np.float16

    def f(k):
        return np.ascontiguousarray(np.asarray(inputs[k], dtype=np.float32))

    hs = f("hs")
    dis = f("dis").reshape(-1)

    xt = np.zeros((3, 4 * N), np.float32)
    bmask = np.zeros((128, 64), np.float32)
    for b, o in enumerate(BAND_ORDER):
        i0, L, e0 = BANDS[o]
        i = np.arange(i0, i0 + L)
        xt[0, b * N + i] = hs[i]
        xt[1, b * N + i] = hs[i + o]
        xt[2, b * N + i] = dis[e0 : e0 + L]
        bm = np.zeros(N, np.float32)
        bm[i] = 1.0
        bmask[:, b * 16 : (b + 1) * 16] = bm.reshape(128, 16)

    def blockdiag(a, b):
        z = np.zeros((128, 128), np.float32)
        z[0:64, 0:64] = a
        z[64:128, 64:128] = b
        return z

    blob16 = np.zeros((3, 10496), f16)
    blob16[:, 0:8192] = xt.astype(f16)
    blob16[0, 8192:10240] = hs.astype(f16)
    blob16[0, 10240:10368] = np.concatenate([f("nW1"), f("eW1")], axis=1)[0].astype(f16)
    blob16[:, 10368:10496] = np.concatenate([f("cW1"), f("kW1")], axis=1).astype(f16)

    blobw16 = np.zeros((128, 420), f16)
    blobw16[:, 0:128] = blockdiag(f("nW2"), f("eW2")).astype(f16)
    blobw16[0:64, 128:129] = f("nW3").astype(f16)      # W3neA (real cols 0:34)
    blobw16[64:128, 130:162] = f("eW3").astype(f16)
    blobw16[0:64, 192:193] = f("nW3").astype(f16)      # W3neB
    blobw16[64:128, 194:226] = f("eW3").astype(f16)
    blobw16[:, 226:354] = blockdiag(f("cW2"), f("kW2")).astype(f16)
    blobw16[0:64, 354:355] = f("cW3").astype(f16)      # W3ckA (real cols 0:2)
    blobw16[64:128, 355:356] = f("kW3").astype(f16)
    blobw16[0:64, 418:419] = f("cW3").astype(f16)      # W3ckB
    blobw16[64:128, 419:420] = f("kW3").astype(f16)

    blob32 = np.zeros((128, 871), np.float32)
    sdn = np.zeros((128, 128), np.float32)
    sup = np.zeros((128, 128), np.float32)
    for q in range(127):
        sdn[q + 1, q] = 1.0  # lhsT: out[m] = v[m+1]
        sup[q, q + 1] = 1.0  # lhsT: out[m] = v[m-1]
    blob32[:, 0:128] = sdn
    blob32[:, 128:256] = sup
    blob32[:, 256:320] = bmask
    blob32[:, 320] = np.concatenate([f("nb1"), f("eb1")])
    blob32[:, 321] = np.concatenate([f("nb2"), f("eb2")])
    blob32[:, 322] = np.concatenate([f("cb1"), f("kb1")])
    blob32[:, 323] = np.concatenate([f("cb2"), f("kb2")])
    blob32[:, 324] = f("cb3")[0]
    blob32[:, 325] = f("kb3")[0]
    blob32[:, 326] = f("nb3")[0]
    blob32[:, 327:359] = f("eb3")[None, :]
    off = 3 * RES
    blob32[:, 359:871] = f("E0")[off : off + N * RES].reshape(128, 512)

    bias8 = np.zeros((128, 8), np.float32)
    bias8[:, 0] = np.concatenate([f("nb1"), f("eb1")])
    bias8[:, 1] = np.concatenate([f("nb2"), f("eb2")])
    bias8[:, 2] = np.concatenate([f("cb1"), f("kb1")])
    bias8[:, 3] = np.concatenate([f("cb2"), f("kb2")])
    bias8[:, 4] = f("cb3")[0]
    bias8[:, 5] = f("kb3")[0]
    bias8[:, 6] = f("nb3")[0]

    return {"blob16": blob16, "blob32": blob32, "blobw16": blobw16,
            "bias8": bias8}


def kernel(**inputs):
    from concourse.bass_utils import run_bass_kernel_spmd

    src = np.asarray(inputs["src"])
    for o, (i0, L, e0) in BANDS.items():
        assert src[e0] == i0 and src[e0 + L - 1] == i0 + L - 1, "unexpected edge order"

    if "nc" not in _CACHE:
        _CACHE["nc"] = _build()
    nc = _CACHE["nc"]

    m = _host_inputs(inputs)
    res = run_bass_kernel_spmd(nc, [m] * 8, core_ids=list(range(8)))
    out = res.results[0]["out"]  # [N*RES, 2] float32
    en = out[:, 0].astype(np.float32) + 1j * out[:, 1].astype(np.float32)
    return en.astype(np.complex64)


# revision 31
# speedup vs baseline: 1.1714x; 1.1714x over previous
"""Trainium2 Bass kernel for nn_Metalayer_sub_62869731279045.

Math: the oracle's edge list is the structured 1-D KNN=2 neighbor graph, so
C = I + Delta and Km are pentadiagonal.  Let D = -Delta and

  G  = wh * (B C + K)            (pentadiagonal, row-diagonals g_o)
  Ninv ~= I + D + D^2 + D^3      (Neumann, band 6)
  T  = Ninv * G - theta*I        (band 8, 17 diagonals, built on device
                                  via banded matrix-matrix products)

Since T is REAL, the expm action needs only a real Taylor chain:
  tau_k = T^k u0,   Uz = e^{i theta} * DX * sum_k (i^k / k!) tau_k
with i^k folded into which accumulator (s_re / s_im) receives each term.
KT=5 terms + fp16 MLP staging give ~1.2e-3 max-rel error vs fp64.

Layouts: length-2048 row vectors live as [128, 16] "fm" tiles (i = 16p+f).
Banded matvec = halo exchange via 2 PE shift-matmuls + one DVE windowed
multiply against 17 stacked diagonal planes + one segmented reduce.
Banded products for the operator assembly use the same windowed-multiply
trick over zero-padded plane tiles (all strides positive by storing the
5-wide D/G plane stacks in reversed diagonal order).

MLPs: c&k edge MLPs run as one 128-wide hidden pipeline over 8192 stacked
edge columns (4 bands); n&e node MLPs as one 128-wide pipeline whose last
layer emits Bd and all 32 Eys rows from a single [128,34] matmul.  Row ->
fm transposes bounce through DRAM with 64B-run descriptor patterns.

All 8 cores run the same single-core program on identical inputs (the
chain is a serial dependency; collectives cost more than they save).
Core 0's output is returned.
"""

import os
import sys
import numpy as np

for _p in ("/opt/trn_rl_repo",):
    if _p not in sys.path:
        sys.path.insert(0, _p)

N = 2048
RES = 32
H = 64
E = 8186
K_WAVE = 2.0 * np.pi / 1.55
WH = 0.75
DX = 1.0 / RES
THETA = 6.234
KT = 3    # Taylor terms (theta-shifted operator norm ~0.2; 3 suffices)
# band order for the stacked ck pipeline: o descending (matches reversed
# diagonal-plane storage so all product access patterns have +strides)
BAND_ORDER = [2, 1, -1, -2]
# (offset o, first valid row i0, edge count L, edge-array start e0)
BANDS = {-2: (2, 2046, 0), -1: (1, 2047, 2046), 1: (0, 2047, 4093), 2: (0, 2046, 6140)}

_CACHE = {}


def _build():
    from contextlib import ExitStack

    import concourse.bass as bass
    import concourse.mybir as mybir
    from concourse import bacc, tile

    f32 = mybir.dt.float32
    f16 = mybir.dt.float16
    AF = mybir.ActivationFunctionType
    ALU = mybir.AluOpType

    nc = bacc.Bacc("TRN2", target_bir_lowering=False, debug=False, num_devices=8)

    def Par(name, shape, dt=f32):
        return nc.declare_dram_parameter(name, list(shape), dt, isOutput=False)

    # consolidated input blobs (3 DMA loads total)
    # blob16 [3, 10496] f16: xt(0:8192) | hs row0 (8192:10240) |
    #   W1ne row0 (10240:10368) | W1ck (10368:10496)
    # blob32 [128, 871] f32: sdn(0:128) | sup(128:256) | bmask(256:320) |
    #   b1ne b2ne b1ck b2ck (320:324) | b3ck(324:326) | nb3(326) |
    #   eb3(327:359) | e0c(359:871)
    # blobw16 [128, 292] f16: W2ne(0:128) | W3neBd(128:130) | W3eys(130:162) |
    #   W2ck(162:290) | W3ckB(290:292); the W3* views feed the direct-fm
    #   matmul transposes (zero-padded to full 128 contraction).
    bias8_d = Par("bias8", [128, 8])
    blob16_d = Par("blob16", [3, 10496], f16)
    blob32_d = Par("blob32", [128, 871])
    blobw16_d = Par("blobw16", [128, 292], f16)
    scratch = nc.dram_tensor("scratch", [43 * N], f32)
    out_d = nc.declare_dram_parameter("out", [N * RES, 2], f32, isOutput=True)
    debug = os.environ.get("KERNEL_DEBUG", "0") == "1"
    dbg_t = {}
    if debug:
        for nm, shape in [
            ("d_eys2b", [128, 512]), ("d_u0", [128, 16]),
            ("d_Dt", [128, 80]), ("d_Gt", [128, 200]), ("d_N1p", [128, 260]),
            ("d_N2p", [128, 340]), ("d_N3p", [128, 420]), ("d_Tpl", [128, 272]),
            ("d_sre", [128, 16]), ("d_sim", [128, 16]),
        ]:
            dbg_t[nm] = nc.dram_tensor(nm, shape, f32)

    TS = bass.ts

    def emit(tc, ctx, pools):
        (consts, work, vec, psA, psC) = pools

        def dma(out_ap, in_ap, eng=None):
            (eng or nc.sync).dma_start(out_ap, in_ap)

        # ---------------- consolidated input loads (3 DMAs) ----------------
        blob16 = consts.tile([3, 10496], f16, tag="blob16")
        bias8 = consts.tile([128, 8], f32, tag="bias8")
        blob32 = consts.tile([128, 871], f32, tag="blob32")
        blobw16 = consts.tile([128, 292], f16, tag="blobw16")
        dma(blob16[:], blob16_d[:], eng=nc.sync)
        dma(bias8[:], bias8_d[:], eng=nc.gpsimd)
        dma(blobw16[:], blobw16_d[:], eng=nc.sync)
        dma(blob32[:], blob32_d[:], eng=nc.gpsimd)
        B32W = 871
        BM = 256  # bmask col base in blob32

        def xt16(q):
            return blob16[:, TS(q, 512)]

        def hs16(q):
            return blob16[0:1, 8192 + q * 512 : 8192 + (q + 1) * 512]

        W1ne = blob16[0:1, 10240:10368]
        W1ck = blob16[:, 10368:10496]
        W2ne = blobw16[:, 0:128]
        W3neBd = blobw16[:, 128:130]
        W3eys = blobw16[:, 130:162]
        W2ck = blobw16[:, 162:290]
        W3ckB = blobw16[:, 290:292]
        sdn = blob32[:, 0:128]
        sup = blob32[:, 128:256]
        bmask = blob32[:, 256:320]
        b1ne = bias8[:, 0:1]
        b2ne = bias8[:, 1:2]
        b1ck = bias8[:, 2:3]
        b2ck = bias8[:, 3:4]
        b3c = bias8[:, 4:5]
        b3k = bias8[:, 5:6]
        nb3 = bias8[:, 6:7]

        # ---------------- persistent SBUF tiles ----------------
        h1ne = consts.tile([128, N], f16, tag="h1ne")
        h2ne = consts.tile([128, N], f16, tag="h2ne")
        h1ck = consts.tile([128, 4 * N], f16, tag="h1ck")
        h2ck = consts.tile([128, 4 * N], f16, tag="h2ck")
        eys2 = consts.tile([128, 16 * RES], f32, tag="eys2")
        eys2b = consts.tile([128, 16 * RES], f32, tag="eys2b")
        u0 = consts.tile([128, 16], f32, tag="u0")
        Dt = consts.tile([128, 80], f32, tag="Dt")        # 5 planes x 16, rev
        Gt = consts.tile([128, 200], f32, tag="Gt")       # 5 planes x 40, rev
        N1p = consts.tile([128, 260], f32, tag="N1p")     # 13 planes x 20
        N2p = consts.tile([128, 340], f32, tag="N2p")     # 17 planes x 20
        N3p = consts.tile([128, 420], f32, tag="N3p")     # 21 planes x 20
        Tpl = consts.tile([128, 272], f32, tag="Tpl")     # col = f*17 + s
        s_re = consts.tile([128, 16], f32, tag="s_re")
        s_im = consts.tile([128, 16], f32, tag="s_im")
        o_int = consts.tile([128, 2 * 16 * RES], f32, tag="o_int")

        AP = bass.AP

        def ap(t, off, dims):
            return AP(t.tensor, t.offset + off, dims)

        tanhc = work.tile([128, 64], f32, tag="tanhc")
        tanhk = work.tile([128, 64], f32, tag="tanhk")
        tb = work.tile([128, 16], f32, tag="tb")
        Bdp = work.tile([128, 16], f32, tag="Bdp")
        Bdp01 = work.tile([128, 16], f32, tag="Bdp01")

        # early memsets of padded plane tiles (no deps -> overlap with MLPs)
        nc.vector.memset(Gt[:], 0.0)
        nc.gpsimd.memset(N1p[:], 0.0)
        nc.vector.memset(N2p[:], 0.0)
        nc.gpsimd.memset(N3p[:], 0.0)
        nc.vector.memset(Dt[:, 32:48], 0.0)
        nc.gpsimd.memset(s_im[:], 0.0)

        # ------- fused ck + ne pipelines (ne chunks interleaved as filler) ----
        # ck: c & k edge MLPs over 4*2048 stacked band columns; ne: n & e node
        # MLPs over 2048 columns.  l3s are partition-stacked (PE psum base
        # must be 0/32/64) so one copy retires two chunks.
        def ck_iter(r):
            ps1 = psA.tile([128, 1024], f32, tag="psA")
            nc.tensor.matmul(ps1[:, 0:512], W1ck, xt16(2 * r))
            nc.tensor.matmul(ps1[:, 512:1024], W1ck, xt16(2 * r + 1))
            nc.scalar.activation(
                h1ck[:, TS(r, 1024)], ps1[:], AF.Relu, bias=b1ck
            )
            ps2 = psA.tile([128, 1024], f32, tag="psA")
            nc.tensor.matmul(ps2[:, 0:512], W2ck, h1ck[:, TS(2 * r, 512)])
            nc.tensor.matmul(ps2[:, 512:1024], W2ck, h1ck[:, TS(2 * r + 1, 512)])
            nc.vector.tensor_scalar(
                h2ck[:, TS(r, 1024)], ps2[:], b2ck, 0.0, ALU.add, op1=ALU.max
            )


        def ne_iter(r):
            ps1 = psA.tile([128, 1024], f32, tag="psA")
            nc.tensor.matmul(ps1[:, 0:512], W1ne, hs16(2 * r))
            nc.tensor.matmul(ps1[:, 512:1024], W1ne, hs16(2 * r + 1))
            nc.scalar.activation(
                h1ne[:, TS(r, 1024)], ps1[:], AF.Relu, bias=b1ne
            )
            ps2 = psA.tile([128, 1024], f32, tag="psA")
            nc.tensor.matmul(ps2[:, 0:512], W2ne, h1ne[:, TS(2 * r, 512)])
            nc.tensor.matmul(ps2[:, 512:1024], W2ne, h1ne[:, TS(2 * r + 1, 512)])
            nc.vector.tensor_scalar(
                h2ne[:, TS(r, 1024)], ps2[:], b2ne, 0.0, ALU.add, op1=ALU.max
            )


        # Direct fm extraction (no DRAM round trip): for a length-2048 row
        # vector v produced as l3 = W3^T @ h2 over columns n, the fm tile
        # value fm[p, f] = v[16p+f] is a matmul with the EDGE-GROUP as output
        # partition:  out[p, c] = sum_h h2[h, 16p+f] * W3[h, c], i.e. lhsT is
        # a stride-16 column view of h2.  16 tiny matmuls fill [128, 32] psum
        # (col 2f+c), then strided tanh reads finish the job.
        def ck_band_direct(b):
            psB = psC.tile([128, 32], f32, tag="psC")
            for fcol in range(16):
                nc.tensor.matmul(
                    psB[:, 2 * fcol : 2 * fcol + 2],
                    ap(h2ck, b * 2048 + fcol, [[8192, 128], [16, 128]]),
                    W3ckB,
                )
            nc.scalar.activation(
                tanhc[:, 16 * b : 16 * b + 16],
                ap(psB, 0, [[32, 128], [2, 16]]),
                AF.Tanh,
                bias=b3c,
            )
            nc.scalar.activation(
                tanhk[:, 16 * b : 16 * b + 16],
                ap(psB, 1, [[32, 128], [2, 16]]),
                AF.Tanh,
                bias=b3k,
            )
            # D plane (reversed: plane j <-> o1; band b -> plane (0,1,3,4)[b])
            # and ascending N1 = I + D plane (band b -> plane (8,7,5,4)[b])
            jD = (0, 1, 3, 4)[b]
            jN = (8, 7, 5, 4)[b]
            for dst_ap in (
                ap(Dt, 16 * jD, [[80, 128], [1, 16]]),
                ap(N1p, jN * 20 + 2, [[260, 128], [1, 16]]),
            ):
                nc.vector.scalar_tensor_tensor(
                    dst_ap,
                    ap(tanhc, 16 * b, [[64, 128], [1, 16]]),
                    -0.1,
                    ap(blob32, BM + 16 * b, [[B32W, 128], [1, 16]]),
                    ALU.mult,
                    ALU.mult,
                )

        def ne_direct():
            # Bd_pre fm via the same trick (W3neBd cols: [nW3;0] | pad)
            psBd = psC.tile([128, 32], f32, tag="psC")
            for fcol in range(16):
                nc.tensor.matmul(
                    psBd[:, 2 * fcol : 2 * fcol + 2],
                    ap(h2ne, fcol, [[2048, 128], [16, 128]]),
                    W3neBd,
                )
            nc.scalar.activation(
                tb[:], ap(psBd, 0, [[32, 128], [2, 16]]), AF.Tanh, bias=nb3
            )
            # eys2[p, r*16+f] = eys[16p+f, r]: per f one [128, 32] matmul
            # (W3eys rows 64:128 = eW3, top half zero)
            psE = psC.tile([128, 512], f32, tag="psC")
            for fcol in range(16):
                nc.tensor.matmul(
                    psE[:, 32 * fcol : 32 * fcol + 32],
                    ap(h2ne, fcol, [[2048, 128], [16, 128]]),
                    W3eys,
                )
            nc.vector.tensor_copy(
                ap(eys2, 0, [[512, 128], [1, 16], [16, 32]]),
                ap(psE, 0, [[512, 128], [32, 16], [1, 32]]),
            )

        ck_iter(0)
        ck_iter(1)
        ne_iter(0)
        ck_iter(2)
        ck_band_direct(0)
        ck_iter(3)
        ck_band_direct(1)
        ck_iter(4)
        ne_iter(1)
        ne_direct()
        ck_iter(5)
        ck_band_direct(2)
        ck_iter(6)
        ck_iter(7)
        ck_band_direct(3)
        # ---------------- diagonal planes ----------------
        # Bd' = wh*K*(2 + 0.5*tanh)
        nc.vector.tensor_scalar(
            Bdp[:], tb[:], 0.5 * K_WAVE * WH, 2.0 * K_WAVE * WH, ALU.mult, op1=ALU.add
        )
        nc.vector.tensor_scalar(Bdp01[:], Bdp[:], 0.1, 0.0, ALU.mult, op1=ALU.add)

        # G planes (reversed, width 40 = 12|16|12), g = (0.1*Bd'*tc + ck2*tk)*mask
        gm4 = work.tile([128, 64], f32, tag="gm4")
        g4 = work.tile([128, 64], f32, tag="g4")
        nc.vector.tensor_tensor(
            ap(gm4, 0, [[64, 128], [16, 4], [1, 16]]),
            ap(tanhc, 0, [[64, 128], [16, 4], [1, 16]]),
            ap(Bdp01, 0, [[16, 128], [0, 4], [1, 16]]),
            ALU.mult,
        )
        nc.vector.scalar_tensor_tensor(
            g4[:], tanhk[:], 0.1 * K_WAVE * WH, gm4[:], ALU.mult, ALU.add
        )
        nc.vector.tensor_tensor(
            ap(Gt, 12, [[200, 128], [40, 2], [1, 16]]),
            ap(g4, 0, [[64, 128], [16, 2], [1, 16]]),
            ap(blob32, BM, [[B32W, 128], [16, 2], [1, 16]]),
            ALU.mult,
        )
        nc.vector.tensor_tensor(
            ap(Gt, 3 * 40 + 12, [[200, 128], [40, 2], [1, 16]]),
            ap(g4, 32, [[64, 128], [16, 2], [1, 16]]),
            ap(blob32, BM + 32, [[B32W, 128], [16, 2], [1, 16]]),
            ALU.mult,
        )
        nc.vector.tensor_copy(Gt[:, 2 * 40 + 12 : 2 * 40 + 28], Bdp[:])
        nc.vector.memset(N1p[:, 6 * 20 + 2 : 6 * 20 + 18], 1.0)

        # halo fill helper: data planes [first..first+n) of a padded tile,
        # plane width w, data at col `pad` of each plane, fill width hw <= pad.
        def halo(t, first, nplanes, w, pad, hw):
            ps = psC.tile([128, 2 * nplanes * hw], f32, tag="psC")
            base = first * w + pad
            # left pads <- v[p-1, f in 16-hw..16] (sup)
            nc.tensor.matmul(
                ps[:, 0 : nplanes * hw],
                sup,
                ap(t, base + 16 - hw, [[t.shape[1], 128], [w, nplanes], [1, hw]]),
            )
            # right pads <- v[p+1, f in 0..hw] (sdn)
            nc.tensor.matmul(
                ps[:, nplanes * hw : 2 * nplanes * hw],
                sdn,
                ap(t, base, [[t.shape[1], 128], [w, nplanes], [1, hw]]),
            )
            nc.vector.tensor_copy(
                ap(
                    t,
                    base - hw,
                    [[t.shape[1], 128], [16 + hw, 2], [w, nplanes], [1, hw]],
                ),
                ap(
                    ps,
                    0,
                    [[2 * nplanes * hw, 128], [nplanes * hw, 2], [hw, nplanes], [1, hw]],
                ),
            )

        halo(Gt, 0, 5, 40, 12, 6)
        halo(N1p, 4, 5, 20, 2, 2)

        # banded product: out_pad.data = D * in_pad  (planes ascending),
        # in0 = Dt (reversed), in1 = in_pad at addr 20*O + 19*j + f + 4.
        # Split over the j (diagonal) axis: DVE takes j 0:3, Pool j 3:5, then
        # DVE adds the Pool partial into the padded output.
        def dprod(in_pad, out_pad, NPo, tag):
            W_in, W_out = in_pad.shape[1], out_pad.shape[1]
            prD = work.tile([128, NPo * 48], f32, tag=tag + "D")
            prD_ap = ap(prD, 0, [[NPo * 48, 128], [48, NPo], [3, 16], [1, 3]])
            nc.vector.tensor_tensor(
                prD_ap,
                ap(Dt, 0, [[80, 128], [0, NPo], [1, 16], [16, 3]]),
                ap(in_pad, 4, [[W_in, 128], [20, NPo], [1, 16], [19, 3]]),
                ALU.mult,
            )
            prP = work.tile([128, NPo * 32], f32, tag=tag + "P")
            prP_ap = ap(prP, 0, [[NPo * 32, 128], [32, NPo], [2, 16], [1, 2]])
            nc.gpsimd.tensor_tensor(
                prP_ap,
                ap(Dt, 48, [[80, 128], [0, NPo], [1, 16], [16, 2]]),
                ap(in_pad, 61, [[W_in, 128], [20, NPo], [1, 16], [19, 2]]),
                ALU.mult,
            )
            tmpP = work.tile([128, NPo * 16], f32, tag=tag + "T")
            nc.gpsimd.tensor_tensor(
                ap(tmpP, 0, [[NPo * 16, 128], [16, NPo], [1, 16]]),
                ap(prP, 0, [[NPo * 32, 128], [32, NPo], [2, 16]]),
                ap(prP, 1, [[NPo * 32, 128], [32, NPo], [2, 16]]),
                ALU.add,
            )
            out_ap = ap(out_pad, 82, [[W_out, 128], [20, NPo], [1, 16]])
            nc.vector.reduce_sum(out_ap, prD_ap, axis=mybir.AxisListType.X)
            nc.vector.tensor_tensor(
                out_ap,
                out_ap,
                ap(tmpP, 0, [[NPo * 16, 128], [16, NPo], [1, 16]]),
                ALU.add,
            )

        dprod(N1p, N2p, 9, "pr2")
        nc.vector.tensor_scalar(
            N2p[:, 8 * 20 + 2 : 8 * 20 + 18],
            N2p[:, 8 * 20 + 2 : 8 * 20 + 18],
            1.0, 0.0, ALU.add, op1=ALU.add,
        )
        halo(N2p, 4, 9, 20, 2, 2)
        dprod(N2p, N3p, 13, "pr3")
        nc.vector.tensor_scalar(
            N3p[:, 10 * 20 + 2 : 10 * 20 + 18],
            N3p[:, 10 * 20 + 2 : 10 * 20 + 18],
            1.0, 0.0, ALU.add, op1=ALU.add,
        )
        # T = N3 * G - theta I   (no halo needed on N3p: read unshifted),
        # j-split DVE (j 0:3) / Pool (j 3:5) like dprod.
        prT = work.tile([128, 17 * 48], f32, tag="prTD")
        prT_ap = ap(prT, 0, [[17 * 48, 128], [48, 17], [3, 16], [1, 3]])
        nc.vector.tensor_tensor(
            prT_ap,
            ap(N3p, 2, [[420, 128], [20, 17], [1, 16], [20, 3]]),
            ap(Gt, 2, [[200, 128], [1, 17], [1, 16], [41, 3]]),
            ALU.mult,
        )
        prTP = work.tile([128, 17 * 32], f32, tag="prTP")
        prTP_ap = ap(prTP, 0, [[17 * 32, 128], [32, 17], [2, 16], [1, 2]])
        nc.gpsimd.tensor_tensor(
            prTP_ap,
            ap(N3p, 62, [[420, 128], [20, 17], [1, 16], [20, 2]]),
            ap(Gt, 125, [[200, 128], [1, 17], [1, 16], [41, 2]]),
            ALU.mult,
        )
        tmpTP = work.tile([128, 272], f32, tag="tmpTP")
        nc.gpsimd.tensor_tensor(
            ap(tmpTP, 0, [[272, 128], [16, 17], [1, 16]]),
            ap(prTP, 0, [[17 * 32, 128], [32, 17], [2, 16]]),
            ap(prTP, 1, [[17 * 32, 128], [32, 17], [2, 16]]),
            ALU.add,
        )
        TplOF = ap(Tpl, 0, [[272, 128], [1, 17], [17, 16]])
        nc.vector.reduce_sum(TplOF, prT_ap, axis=mybir.AxisListType.X)
        nc.vector.tensor_tensor(
            TplOF, TplOF, ap(tmpTP, 0, [[272, 128], [16, 17], [1, 16]]), ALU.add
        )
        nc.vector.tensor_scalar(
            ap(Tpl, 8, [[272, 128], [17, 16]]),
            ap(Tpl, 8, [[272, 128], [17, 16]]),
            -THETA, 0.0, ALU.add, op1=ALU.add,
        )

        # eys2b = eys2 + eb3 (per-r bias)
        nc.vector.tensor_tensor(
            ap(eys2b, 0, [[512, 128], [16, 32], [1, 16]]),
            ap(eys2, 0, [[512, 128], [16, 32], [1, 16]]),
            ap(blob32, 327, [[B32W, 128], [1, 32], [0, 16]]),
            ALU.add,
        )
        # u0[i] = sum_r eys2b[i,r] * e0c[i,r]
        pu = work.tile([128, 16 * RES], f32, tag="pu")
        nc.vector.tensor_tensor(
            ap(pu, 0, [[512, 128], [32, 16], [1, 32]]),
            ap(eys2b, 0, [[512, 128], [1, 16], [16, 32]]),
            ap(blob32, 359, [[B32W, 128], [32, 16], [1, 32]]),
            ALU.mult,
        )
        nc.vector.reduce_sum(
            u0[:],
            ap(pu, 0, [[512, 128], [32, 16], [1, 32]]),
            axis=mybir.AxisListType.X,
        )


        # ---------------- real Taylor chain ----------------
        t_cur = vec.tile([128, 32], f32, tag="vec")
        nc.vector.memset(t_cur[:], 0.0)
        nc.vector.tensor_copy(t_cur[:, 8:24], u0[:])
        nc.vector.tensor_scalar(
            s_re[:], u0[:], DX, 0.0, ALU.mult, op1=ALU.add
        )
        fact = 1.0
        for k in range(1, KT + 1):
            psh = psC.tile([128, 16], f32, tag="psC")
            nc.tensor.matmul(psh[:, 0:8], sup, t_cur[:, 16:24])
            nc.tensor.matmul(psh[:, 8:16], sdn, t_cur[:, 8:16])
            nc.vector.tensor_copy(
                ap(t_cur, 0, [[32, 128], [24, 2], [1, 8]]),
                ap(psh, 0, [[16, 128], [8, 2], [1, 8]]),
            )
            pr = work.tile([128, 272], f32, tag="prc")
            pr_ap = ap(pr, 0, [[272, 128], [17, 16], [1, 17]])
            nc.vector.tensor_tensor(
                pr_ap,
                ap(t_cur, 0, [[32, 128], [1, 16], [1, 17]]),
                ap(Tpl, 0, [[272, 128], [17, 16], [1, 17]]),
                ALU.mult,
            )
            t_nxt = vec.tile([128, 32], f32, tag="vec")
            nc.vector.reduce_sum(
                ap(t_nxt, 8, [[32, 128], [1, 16]]), pr_ap, axis=mybir.AxisListType.X
            )
            fact *= k
            coef = DX / fact * (-1.0 if k % 4 in (2, 3) else 1.0)
            dst = s_im if k % 2 == 1 else s_re
            nc.vector.scalar_tensor_tensor(
                dst[:], t_nxt[:, 8:24], coef, dst[:], ALU.mult, ALU.add
            )
            t_cur = t_nxt

        # ---------------- Uz = e^{i theta} s;  En = Uz * Eys ----------------
        cth, sth = float(np.cos(THETA)), float(np.sin(THETA))
        uzr = work.tile([128, 16], f32, tag="uzr")
        uzi = work.tile([128, 16], f32, tag="uzi")
        p1 = work.tile([128, 16], f32, tag="p1")
        nc.vector.tensor_scalar(p1[:], s_im[:], sth, 0.0, ALU.mult, op1=ALU.add)
        nc.vector.scalar_tensor_tensor(
            uzr[:], s_re[:], cth, p1[:], ALU.mult, ALU.subtract
        )
        nc.vector.tensor_scalar(p1[:], s_re[:], sth, 0.0, ALU.mult, op1=ALU.add)
        nc.vector.scalar_tensor_tensor(
            uzi[:], s_im[:], cth, p1[:], ALU.mult, ALU.add
        )
        if debug:
            for nm, t in [
                ("d_eys2b", eys2b),
                ("d_u0", u0), ("d_Dt", Dt), ("d_Gt", Gt), ("d_N1p", N1p),
                ("d_N2p", N2p), ("d_N3p", N3p), ("d_Tpl", Tpl), ("d_sre", s_re),
                ("d_sim", s_im),
            ]:
                nc.sync.dma_start(dbg_t[nm][:], t[:])
        # interleave En = Uz * Eys into o_int and stream out in two halves so
        # the first DMA overlaps the second half's compute
        for half in range(2):
            for c, uz in ((0, uzr), (1, uzi)):
                nc.vector.tensor_tensor(
                    ap(o_int, 512 * half + c, [[1024, 128], [64, 8], [2, 32]]),
                    ap(eys2b, 8 * half, [[512, 128], [1, 8], [16, 32]]),
                    ap(uz, 8 * half, [[16, 128], [1, 8], [0, 32]]),
                    ALU.mult,
                )
            (nc.sync if half == 0 else nc.gpsimd).dma_start(
                AP(out_d, 512 * half, [[1024, 128], [1, 512]]),
                o_int[:, TS(half, 512)],
            )

    with tile.TileContext(nc) as tc:
        ctx = ExitStack()
        try:
            pools = (
                ctx.enter_context(tc.tile_pool(name="consts", bufs=1)),
                ctx.enter_context(tc.tile_pool(name="work", bufs=2)),
                ctx.enter_context(tc.tile_pool(name="vec", bufs=3)),
                ctx.enter_context(tc.tile_pool(name="psA", bufs=3, space="PSUM")),
                ctx.enter_context(tc.tile_pool(name="psC", bufs=2, space="PSUM")),
            )
            emit(tc, ctx, pools)
        finally:
            ctx.close()

    nc.compile()
    nc.finalize()
    return nc


def _host_inputs(inputs):
    """Stage the oracle's inputs into the kernel's DRAM parameters."""
    f16 = np.float16

    def f(k):
        return np.ascontiguousarray(np.asarray(inputs[k], dtype=np.float32))

    hs = f("hs")
    dis = f("dis").reshape(-1)

    xt = np.zeros((3, 4 * N), np.float32)
    bmask = np.zeros((128, 64), np.float32)
    for b, o in enumerate(BAND_ORDER):
        i0, L, e0 = BANDS[o]
        i = np.arange(i0, i0 + L)
        xt[0, b * N + i] = hs[i]
        xt[1, b * N + i] = hs[i + o]
        xt[2, b * N + i] = dis[e0 : e0 + L]
        bm = np.zeros(N, np.float32)
        bm[i] = 1.0
        bmask[:, b * 16 : (b + 1) * 16] = bm.reshape(128, 16)

    def blockdiag(a, b):
        z = np.zeros((128, 128), np.float32)
        z[0:64, 0:64] = a
        z[64:128, 64:128] = b
        return z

    blob16 = np.zeros((3, 10496), f16)
    blob16[:, 0:8192] = xt.astype(f16)
    blob16[0, 8192:10240] = hs.astype(f16)
    blob16[0, 10240:10368] = np.concatenate([f("nW1"), f("eW1")], axis=1)[0].astype(f16)
    blob16[:, 10368:10496] = np.concatenate([f("cW1"), f("kW1")], axis=1).astype(f16)

    blobw16 = np.zeros((128, 292), f16)
    blobw16[:, 0:128] = blockdiag(f("nW2"), f("eW2")).astype(f16)
    blobw16[0:64, 128:129] = f("nW3").astype(f16)      # W3neBd col 0
    blobw16[64:128, 130:162] = f("eW3").astype(f16)    # W3eys rows 64:128
    blobw16[:, 162:290] = blockdiag(f("cW2"), f("kW2")).astype(f16)
    blobw16[0:64, 290:291] = f("cW3").astype(f16)      # W3ckB
    blobw16[64:128, 291:292] = f("kW3").astype(f16)

    blob32 = np.zeros((128, 871), np.float32)
    sdn = np.zeros((128, 128), np.float32)
    sup = np.zeros((128, 128), np.float32)
    for q in range(127):
        sdn[q + 1, q] = 1.0  # lhsT: out[m] = v[m+1]
        sup[q, q + 1] = 1.0  # lhsT: out[m] = v[m-1]
    blob32[:, 0:128] = sdn
    blob32[:, 128:256] = sup
    blob32[:, 256:320] = bmask
    blob32[:, 320] = np.concatenate([f("nb1"), f("eb1")])
    blob32[:, 321] = np.concatenate([f("nb2"), f("eb2")])
    blob32[:, 322] = np.concatenate([f("cb1"), f("kb1")])
    blob32[:, 323] = np.concatenate([f("cb2"), f("kb2")])
    blob32[:, 324] = f("cb3")[0]
    blob32[:, 325] = f("kb3")[0]
    blob32[:, 326] = f("nb3")[0]
    blob32[:, 327:359] = f("eb3")[None, :]
    off = 3 * RES
    blob32[:, 359:871] = f("E0")[off : off + N * RES].reshape(128, 512)

    bias8 = np.zeros((128, 8), np.float32)
    bias8[:, 0] = np.concatenate([f("nb1"), f("eb1")])
    bias8[:, 1] = np.concatenate([f("nb2"), f("eb2")])
    bias8[:, 2] = np.concatenate([f("cb1"), f("kb1")])
    bias8[:, 3] = np.concatenate([f("cb2"), f("kb2")])
    bias8[:, 4] = f("cb3")[0]
    bias8[:, 5] = f("kb3")[0]
    bias8[:, 6] = f("nb3")[0]

    return {"blob16": blob16, "blob32": blob32, "blobw16": blobw16,
            "bias8": bias8}


def kernel(**inputs):
    from concourse.bass_utils import run_bass_kernel_spmd

    src = np.asarray(inputs["src"])
    for o, (i0, L, e0) in BANDS.items():
        assert src[e0] == i0 and src[e0 + L - 1] == i0 + L - 1, "unexpected edge order"

    if "nc" not in _CACHE:
        _CACHE["nc"] = _build()
    nc = _CACHE["nc"]

    m = _host_inputs(inputs)
    res = run_bass_kernel_spmd(nc, [m] * 8, core_ids=list(range(8)))
    out = res.results[0]["out"]  # [N*RES, 2] float32
    en = out[:, 0].astype(np.float32) + 1j * out[:, 1].astype(np.float32)
    return en.astype(np.complex64)
